# revision 1
# baseline (speedup 1.0000x reference)
"""Trainium2 Bass kernel for nn_DPFlashAttention (B=4, S=2048, E=2048, H=16).

Sharding: 8 cores = 4 batches (data-parallel) x 2 head-groups (tensor-parallel
over heads). Core c handles batch c//2, heads (c%2)*8 .. (c%2)*8+8.

v2: fp8 pipeline. The DP noise (sigma=4.85) dominates the attention output
(ctx sigma~0.036), so the attention path tolerates fp8:
  P1  q/k projections in fp8e4 DoubleRow (K=256/pass), resident SBUF out
  P2  v projection in fp8e4 DoubleRow, resident SBUF out
  P3  per head: scores in plain fp8 (D=128 contraction), exp with constant
      shift -C (cancels in normalization; keeps e4m3 weights <= ~200),
      attn weights quantized e4m3, attn@V in DoubleRow (keys paired),
      denominators via DVE sums + ones-column f32r matmul, DP noise add
  P4  out^T partial = Wo_shard @ (ctx + noise) in bf16 (noise needs >=bf16)
Host: pre-transposes + pre-quantizes per-batch inputs (fp8e4) and weights,
pre-scales noise by the DP sigma (bf16), sums head-group partials,
transposes back, adds bo.
"""
import math
import sys

sys.path.insert(0, "/opt/trn_rl_repo")

import numpy as np

import concourse.bass as bass
import concourse.mybir as mybir
import concourse.tile as tile
from concourse.vector_clock import ScopedClock


class TileContextFixed(tile.TileContext):
    """This walrus build caps sync waits per instruction; split the closing
    drain's waits across single-wait NoOps (same engine => same semantics)."""

    def _drain_and_barrier(self, tick_clock, wait_clock):
        carrier = self.nc.sync.nop(nofuse=True, hint="drain_waits")
        wait_clock.add_sem_waits(
            carrier.ins, ScopedClock({None: tick_clock.global_clock})
        )
        si = carrier.ins.sync_info
        waits = list(si.on_wait) if si is not None else []
        if si is not None:
            si.on_wait[:] = waits[:1]
        for w in waits[1:]:
            n = self.nc.sync.nop(nofuse=True, hint="drain_waits")
            n.ins.sync_info = mybir.SyncInfo(on_wait=[w], on_update=[])
        self.nc.sync.drain()
        self.nc.all_engine_barrier()
        assert self.sems is not None
        popped = self.nc._tile_sem_poison_stack.pop()
        assert popped is self._sem_poison
        self.nc.clear_and_free_semaphores(list(self.sems.allocated().values()))
        self.nc.all_engine_barrier()


def split_excess_waits(nc, opcodes=None, cap=1):
    """Hoist waits beyond `cap` onto same-engine NoOps placed just before the
    instruction; engine queues execute in order so blocking is preserved."""
    n_split = 0
    for fn in nc.m.functions:
        for blk in fn.blocks:
            new = []
            for inst in blk.instructions:
                si = inst.sync_info
                if (
                    (opcodes is None or inst.opcode in opcodes)
                    and si is not None
                    and len(si.on_wait) > cap
                ):
                    waits = list(si.on_wait)
                    for j, w in enumerate(waits[cap:]):
                        nop = mybir.InstNoOp(
                            name=f"{inst.name}-w{j}", engine=inst.engine
                        )
                        nop.sync_info = mybir.SyncInfo(on_wait=[w], on_update=[])
                        new.append(nop)
                        n_split += 1
                    si.on_wait[:] = waits[:cap]
                new.append(inst)
            blk.instructions[:] = new
    return n_split

F32 = mybir.dt.float32
F32R = mybir.dt.float32r
BF16 = mybir.dt.bfloat16
FP8 = mybir.dt.float8e4
FP8W = mybir.dt.float8e5   # attn weights: e5m2 spans exp(+-9) w/o subnormals
AF = mybir.ActivationFunctionType
DR = mybir.MatmulPerfMode.DoubleRow

S = 2048
E = 2048
EG = 1024          # per-core e_out shard (8 heads x 128)
D = 128
NHEAD = 8          # heads per core
SCALE = 1.0 / math.sqrt(128.0)
INT8 = mybir.dt.int8
# DVE bit-trick exp -> e5m2 bits: round(z*(4/ln2) + 60) = exp(z) in e5m2.
# Uniform half-LSB bias from f32->int8 rounding cancels in softmax norm.
EXP_A = SCALE * 4.0 / math.log(2.0)
EXP_B = 60.0


def build_kernel_nc(phases=4, repeat=1):
    nc = bass.Bass()

    xq = nc.dram_tensor("xq8", [E, S], FP8, kind="ExternalInput")
    xk = nc.dram_tensor("xk8", [E, S], FP8, kind="ExternalInput")
    xv = nc.dram_tensor("xv8", [E, S], FP8, kind="ExternalInput")
    wq = nc.dram_tensor("wq8", [E, EG], FP8, kind="ExternalInput")
    wk = nc.dram_tensor("wk8", [E, EG], FP8, kind="ExternalInput")
    wv = nc.dram_tensor("wv8", [E, EG], FP8, kind="ExternalInput")
    wo = nc.dram_tensor("wob", [EG, E], BF16, kind="ExternalInput")
    bq = nc.dram_tensor("bq2", [128, 8], F32, kind="ExternalInput")
    bk = nc.dram_tensor("bk2", [128, 8], F32, kind="ExternalInput")
    bv = nc.dram_tensor("bvb", [128, EG], F32, kind="ExternalInput")
    noi = nc.dram_tensor("noiseT", [EG, S], BF16, kind="ExternalInput")
    out = nc.dram_tensor("outT", [E, S], F32, kind="ExternalOutput")

    for _rep in range(repeat):
        with TileContextFixed(nc) as tc, \
             nc.allow_low_precision(reason="fp8 attention path is intended"):
            with tc.tile_pool(name="const", bufs=1) as cpool:
                bq_sb = cpool.tile([128, 8], F32, tag="bq")
                nc.sync.dma_start(bq_sb[:], bq[:])
                bk_sb = cpool.tile([128, 8], F32, tag="bk")
                nc.sync.dma_start(bk_sb[:], bk[:])
                bv_sb = cpool.tile([128, EG], F32, tag="bv")
                nc.sync.dma_start(bv_sb[:], bv[:])
                ones2 = cpool.tile([128, 2, 128], FP8W, tag="ones2")
                nc.vector.memset(ones2[:], 1.0)

                with tc.tile_pool(name="res", bufs=1) as rpool:
                    q_sb = rpool.tile([128, NHEAD, S], FP8, tag="q")
                    k_sb = rpool.tile([128, NHEAD, S], FP8, tag="k")
                    v_sb = rpool.tile([128, 16, EG], FP8, tag="v")
                    ctx_sb = rpool.tile([128, NHEAD, S], BF16, tag="ctx")

                    # ------------ P1: q/k projections (feature-major out) -------
                    with tc.tile_pool(name="p1x", bufs=2) as xpool, \
                         tc.tile_pool(name="p1w", bufs=2) as wpool, \
                         tc.tile_pool(name="p1ps", bufs=8, space="PSUM") as pspool:
                        for (xin, win, bsb, dst) in (
                            (xq, wq, bq_sb, q_sb),
                            (xk, wk, bk_sb, k_sb),
                        ):
                            xall = xpool.tile([128, 8, 2, S], FP8, tag="x")
                            nc.sync.dma_start(
                                xall[:],
                                xin.rearrange(
                                    "(kt two p) n -> p kt two n", p=128, two=2
                                ),
                            )
                            for m in range(8):
                                wm = wpool.tile([128, 8, 2, 128], FP8, tag="w")
                                nc.sync.dma_start(
                                    wm[:],
                                    win[:, m * 128:(m + 1) * 128].rearrange(
                                        "(kt two p) m -> p kt two m", p=128, two=2
                                    ),
                                )
                                for n2 in range(4):
                                    ps = pspool.tile([128, 512], F32, tag="ps")
                                    for h2 in range(2):
                                        n = n2 * 2 + h2
                                        for kt in range(8):
                                            nc.tensor.matmul(
                                                ps[:, h2 * 256:(h2 + 1) * 256],
                                                wm[:, kt],
                                                xall[:, kt, :, n * 256:(n + 1) * 256],
                                                start=(kt == 0),
                                                stop=(kt == 7),
                                                perf_mode=DR,
                                            )
                                    nc.vector.tensor_scalar_add(
                                        dst[:, m, n2 * 512:(n2 + 1) * 512],
                                        ps[:],
                                        bsb[:, m:m + 1],
                                    )

                    # ------------ P2: v projection (natural [s, e_out]) ---------
                    if phases < 2:
                        return nc, 0
                    with tc.tile_pool(name="p2w", bufs=1) as wvpool, \
                         tc.tile_pool(name="p2x", bufs=2) as xvpool, \
                         tc.tile_pool(name="p2ps", bufs=8, space="PSUM") as pspool:
                        wvsb = wvpool.tile([128, 8, 2, EG], FP8, tag="wv")
                        nc.sync.dma_start(
                            wvsb[:],
                            wv.rearrange("(kt two p) m -> p kt two m", p=128, two=2),
                        )
                        for m in range(16):
                            xm = xvpool.tile([128, 8, 2, 128], FP8, tag="xv")
                            nc.sync.dma_start(
                                xm[:],
                                xv[:, m * 128:(m + 1) * 128].rearrange(
                                    "(kt two p) s -> p kt two s", p=128, two=2
                                ),
                            )
                            for n2 in range(2):
                                ps = pspool.tile([128, 512], F32, tag="psv")
                                for h2 in range(2):
                                    nn = n2 * 2 + h2
                                    for kt in range(8):
                                        nc.tensor.matmul(
                                            ps[:, h2 * 256:(h2 + 1) * 256],
                                            xm[:, kt],
                                            wvsb[:, kt, :, nn * 256:(nn + 1) * 256],
                                            start=(kt == 0),
                                            stop=(kt == 7),
                                            perf_mode=DR,
                                        )
                                nc.vector.tensor_add(
                                    v_sb[:, m, n2 * 512:(n2 + 1) * 512],
                                    ps[:],
                                    bv_sb[:, n2 * 512:(n2 + 1) * 512],
                                )

                    # ------------ P3: attention, resident ctx -------------------
                    if phases < 3:
                        return nc, 0
                    with tc.tile_pool(name="p3p", bufs=2) as ppool, \
                         tc.tile_pool(name="p3n", bufs=2) as npool, \
                         tc.tile_pool(name="p3s", bufs=2) as spool, \
                         tc.tile_pool(name="psS", bufs=4, space="PSUM") as psS, \
                         tc.tile_pool(name="psC", bufs=1, space="PSUM") as psC, \
                         tc.tile_pool(name="psZ", bufs=1, space="PSUM") as psZ:
                        for h in range(NHEAD):
                            for qc in range(4):
                                q0 = qc * 512
                                # one PSUM bank (2KB) per accumulation chain:
                                # start=True zeroes the whole bank, so concurrent
                                # chains must never share one. plane j = chain j.
                                ps_ctx = psC.tile([128, 2, 512], F32, tag="ctxps")
                                ps_z = psZ.tile([128, 2, 512], F32, tag="zps")
                                for kt2 in range(8):
                                    psb = ppool.tile([128, 2, 512], FP8W, tag="p")
                                    for half in range(2):
                                        kc = kt2 * 2 + half
                                        ps_s = psS.tile([128, 512], F32, tag="sps")
                                        nc.tensor.matmul(
                                            ps_s[:],
                                            k_sb[:, h, kc * 128:(kc + 1) * 128],
                                            q_sb[:, h, q0:q0 + 512],
                                            start=True,
                                            stop=True,
                                        )
                                        nc.scalar.activation(
                                            psb[:, half], ps_s[:], AF.Exp,
                                            scale=SCALE,
                                        )
                                    for j in range(2):
                                        nc.tensor.matmul(
                                            ps_ctx[:, j, 0:256],
                                            v_sb[:, 2 * kt2:2 * kt2 + 2,
                                                 h * 128:(h + 1) * 128],
                                            psb[:, :, j * 256:(j + 1) * 256],
                                            start=(kt2 == 0),
                                            stop=(kt2 == 7),
                                            perf_mode=DR,
                                        )
                                        nc.tensor.matmul(
                                            ps_z[:, j, 0:256],
                                            ones2[:],
                                            psb[:, :, j * 256:(j + 1) * 256],
                                            start=(kt2 == 0),
                                            stop=(kt2 == 7),
                                            perf_mode=DR,
                                        )
                                # normalize + noise into resident ctx
                                nsb = npool.tile([128, 512], BF16, tag="n")
                                nc.sync.dma_start(
                                    nsb[:],
                                    noi[h * 128:(h + 1) * 128, q0:q0 + 512],
                                )
                                rb_sb = spool.tile([128, 512], F32, tag="rb")
                                nc.vector.reciprocal(rb_sb[:], ps_z[:, :, 0:256])
                                tmp = spool.tile([128, 512], F32, tag="tmp")
                                nc.vector.tensor_mul(
                                    tmp[:], ps_ctx[:, :, 0:256], rb_sb[:]
                                )
                                nc.vector.tensor_add(
                                    ctx_sb[:, h, q0:q0 + 512],
                                    tmp[:],
                                    nsb[:],
                                )

                    # ------------ P4: out projection (bf16) ---------------------
                    if phases < 4:
                        return nc, 0
                    with tc.tile_pool(name="p4w", bufs=2) as wpool4, \
                         tc.tile_pool(name="p4o", bufs=4) as opool, \
                         tc.tile_pool(name="p4ps", bufs=8, space="PSUM") as pspool:
                        for m in range(16):
                            wosb = wpool4.tile([128, NHEAD, 128], BF16, tag="wo")
                            nc.sync.dma_start(
                                wosb[:],
                                wo[:, m * 128:(m + 1) * 128].rearrange(
                                    "(kt p) n -> p kt n", p=128
                                ),
                            )
                            for n in range(4):
                                ps = pspool.tile([128, 512], F32, tag="pso")
                                for kt in range(NHEAD):
                                    nc.tensor.matmul(
                                        ps[:],
                                        wosb[:, kt],
                                        ctx_sb[:, kt, n * 512:(n + 1) * 512],
                                        start=(kt == 0),
                                        stop=(kt == NHEAD - 1),
                                    )
                                osb = opool.tile([128, 512], F32, tag="oo")
                                nc.scalar.copy(osb[:], ps[:])
                                nc.sync.dma_start(
                                    out[m * 128:(m + 1) * 128,
                                        n * 512:(n + 1) * 512],
                                    osb[:],
                                )
    n = split_excess_waits(nc)
    return nc, n


B = 4
NOISE_SCALE = 1.0 * math.sqrt(2.0 * math.log(1.25 / 1e-05)) / 1.0


def _make_in_maps(query, key_t, value, Wq, bq, Wk, bk, Wv, bv, Wo, bo, noise):
    import ml_dtypes

    E4 = ml_dtypes.float8_e4m3
    BF = ml_dtypes.bfloat16
    WqT = np.asarray(Wq, np.float32).T.astype(E4)
    WkT = np.asarray(Wk, np.float32).T.astype(E4)
    WvT = np.asarray(Wv, np.float32).T.astype(E4)
    WoT = np.asarray(Wo, np.float32).T.astype(BF)
    bq = np.asarray(bq, np.float32)
    bk = np.asarray(bk, np.float32)
    bv = np.asarray(bv, np.float32)
    xts = {}
    for b in range(B):
        xts[b] = (
            np.ascontiguousarray(np.asarray(query[b], np.float32).T).astype(E4),
            np.ascontiguousarray(np.asarray(key_t[b], np.float32).T).astype(E4),
            np.ascontiguousarray(np.asarray(value[b], np.float32).T).astype(E4),
        )
    in_maps = []
    for c in range(8):
        b, g = c // 2, c % 2
        cols = slice(g * EG, (g + 1) * EG)
        in_maps.append({
            "xq8": xts[b][0],
            "xk8": xts[b][1],
            "xv8": xts[b][2],
            "wq8": np.ascontiguousarray(WqT[:, cols]),
            "wk8": np.ascontiguousarray(WkT[:, cols]),
            "wv8": np.ascontiguousarray(WvT[:, cols]),
            "wob": np.ascontiguousarray(WoT[cols, :]),
            "bq2": np.ascontiguousarray(bq[cols].reshape(8, 128).T),
            "bk2": np.ascontiguousarray(bk[cols].reshape(8, 128).T),
            "bvb": np.ascontiguousarray(
                np.broadcast_to(bv[cols][None, :], (128, EG))
            ),
            "noiseT": np.ascontiguousarray(
                (np.asarray(noise[b], np.float32)[:, cols].T * NOISE_SCALE)
            ).astype(BF),
        })
    return in_maps


def kernel(**inputs) -> np.ndarray:
    from concourse.bass_utils import run_bass_kernel_spmd

    nc, _ = build_kernel_nc()
    in_maps = _make_in_maps(**inputs)
    res = run_bass_kernel_spmd(nc, in_maps, core_ids=list(range(8)))
    bo = np.asarray(inputs["bo"], np.float32)
    out = np.empty((B, S, E), np.float32)
    for b in range(B):
        p0 = res.results[2 * b]["outT"]
        p1 = res.results[2 * b + 1]["outT"]
        out[b] = (p0 + p1).T + bo[None, :]
    return out



# revision 2
# speedup vs baseline: 1.1058x; 1.1058x over previous
"""Trainium2 Bass kernel for nn_DPFlashAttention (B=4, S=2048, E=2048, H=16).

Sharding: 8 cores = 4 batches (data-parallel) x 2 head-groups (tensor-parallel
over heads). Core c handles batch c//2, heads (c%2)*8 .. (c%2)*8+8.

v3: N_out=512 everywhere + software-pipelined P3.
All DoubleRow matmuls emit a full PSUM bank (N_out=512, rhs free 1024) so the
256-col LDWEIGHTS (213ns @1.2GHz) hides under the ~241ns fill; at the old
N_out=256 the weight load dominated (213 vs 120ns).
P3 runs a 1-deep software pipeline: scores+exp of (h,qc) iter i interleave
with attn@V + Z(ones) matmuls of iter i-1 in PE program order, so the PE
never waits on exp. exp is split per kt2 pair across the Act engine (native
Exp -> e5m2) and DVE (bit-trick: e5m2 bits = trunc(z*4/ln2 + 60) as int8;
the uniform half-LSB truncation bias cancels in softmax normalization).
  P1  q/k projections fp8e4 DR, bias via Act Identity(bias) -> resident SBUF
  P2  v projection fp8e4 DR, bias via DVE tensor_add (per-column bias)
  P3  per (h,qc): 16 score MMs (fp8, K=128, N=512), 8 exp pairs (Act 5/DVE 3),
      8 AV MMs + 8 Z MMs (DR, N_out=512), normalize = DVE recip+mul,
      noise add -> resident bf16 ctx
  P4  out^T partial = Wo_shard @ (ctx + noise) in bf16, bf16 DMA out
Host: pre-transposes + pre-quantizes per-batch inputs (fp8e4) and weights,
pre-scales noise by the DP sigma (bf16), sums head-group partials,
transposes back, adds bo.
"""
import math
import sys

sys.path.insert(0, "/opt/trn_rl_repo")

import numpy as np

import concourse.bass as bass
import concourse.mybir as mybir
import concourse.tile as tile
from concourse.vector_clock import ScopedClock


class TileContextFixed(tile.TileContext):
    """This walrus build caps sync waits per instruction; split the closing
    drain's waits across single-wait NoOps (same engine => same semantics)."""

    def _drain_and_barrier(self, tick_clock, wait_clock):
        carrier = self.nc.sync.nop(nofuse=True, hint="drain_waits")
        wait_clock.add_sem_waits(
            carrier.ins, ScopedClock({None: tick_clock.global_clock})
        )
        si = carrier.ins.sync_info
        waits = list(si.on_wait) if si is not None else []
        if si is not None:
            si.on_wait[:] = waits[:1]
        for w in waits[1:]:
            n = self.nc.sync.nop(nofuse=True, hint="drain_waits")
            n.ins.sync_info = mybir.SyncInfo(on_wait=[w], on_update=[])
        self.nc.sync.drain()
        self.nc.all_engine_barrier()
        assert self.sems is not None
        popped = self.nc._tile_sem_poison_stack.pop()
        assert popped is self._sem_poison
        self.nc.clear_and_free_semaphores(list(self.sems.allocated().values()))
        self.nc.all_engine_barrier()


def split_excess_waits(nc, opcodes=None, cap=1):
    """Hoist waits beyond `cap` onto same-engine NoOps placed just before the
    instruction; engine queues execute in order so blocking is preserved."""
    n_split = 0
    for fn in nc.m.functions:
        for blk in fn.blocks:
            new = []
            for inst in blk.instructions:
                si = inst.sync_info
                if (
                    (opcodes is None or inst.opcode in opcodes)
                    and si is not None
                    and len(si.on_wait) > cap
                ):
                    waits = list(si.on_wait)
                    for j, w in enumerate(waits[cap:]):
                        nop = mybir.InstNoOp(
                            name=f"{inst.name}-w{j}", engine=inst.engine
                        )
                        nop.sync_info = mybir.SyncInfo(on_wait=[w], on_update=[])
                        new.append(nop)
                        n_split += 1
                    si.on_wait[:] = waits[:cap]
                new.append(inst)
            blk.instructions[:] = new
    return n_split

F32 = mybir.dt.float32
F32R = mybir.dt.float32r
BF16 = mybir.dt.bfloat16
FP8 = mybir.dt.float8e4
FP8W = mybir.dt.float8e5   # attn weights: e5m2 spans exp(+-9) w/o subnormals
INT8 = mybir.dt.int8
AF = mybir.ActivationFunctionType
ALU = mybir.AluOpType
DR = mybir.MatmulPerfMode.DoubleRow

S = 2048
E = 2048
EG = 1024          # per-core e_out shard (8 heads x 128)
D = 128
NHEAD = 8          # heads per core
SCALE = 1.0 / math.sqrt(128.0)
# DVE bit-trick exp -> e5m2 bits: trunc(z*(4/ln2)*SCALE + 60) = exp(z*SCALE)
# in e5m2 (x 2^-1/8 uniform bias that cancels in softmax normalization).
EXP_A = SCALE * 4.0 / math.log(2.0)
EXP_B = 60.0
ACT_KT2 = (0, 2, 4, 6, 7)  # exp pairs on Act engine; rest on DVE


def build_kernel_nc(phases=4, repeat=1):
    nc = bass.Bass()

    xq = nc.dram_tensor("xq8", [E, S], FP8, kind="ExternalInput")
    xk = nc.dram_tensor("xk8", [E, S], FP8, kind="ExternalInput")
    xv = nc.dram_tensor("xv8", [E, S], FP8, kind="ExternalInput")
    wq = nc.dram_tensor("wq8", [E, EG], FP8, kind="ExternalInput")
    wk = nc.dram_tensor("wk8", [E, EG], FP8, kind="ExternalInput")
    wv = nc.dram_tensor("wv8", [E, EG], FP8, kind="ExternalInput")
    wo = nc.dram_tensor("wob", [EG, E], BF16, kind="ExternalInput")
    bq = nc.dram_tensor("bq2", [128, 8], F32, kind="ExternalInput")
    bk = nc.dram_tensor("bk2", [128, 8], F32, kind="ExternalInput")
    bv = nc.dram_tensor("bvb", [128, EG], F32, kind="ExternalInput")
    noi = nc.dram_tensor("noiseT", [EG, S], BF16, kind="ExternalInput")
    out = nc.dram_tensor("outT", [E, S], BF16, kind="ExternalOutput")

    for _rep in range(repeat):
        with TileContextFixed(nc) as tc, \
             nc.allow_low_precision(reason="fp8 attention path is intended"):
            with tc.tile_pool(name="const", bufs=1) as cpool:
                bq_sb = cpool.tile([128, 8], F32, tag="bq")
                nc.sync.dma_start(bq_sb[:], bq[:])
                bk_sb = cpool.tile([128, 8], F32, tag="bk")
                nc.sync.dma_start(bk_sb[:], bk[:])
                bv_sb = cpool.tile([128, EG], F32, tag="bv")
                nc.sync.dma_start(bv_sb[:], bv[:])
                ones2 = cpool.tile([128, 2, 128], FP8W, tag="ones2")
                nc.vector.memset(ones2[:], 1.0)

                with tc.tile_pool(name="res", bufs=1) as rpool:
                    q_sb = rpool.tile([128, NHEAD, S], FP8, tag="q")
                    k_sb = rpool.tile([128, NHEAD, S], FP8, tag="k")
                    v_sb = rpool.tile([128, 16, EG], FP8, tag="v")
                    ctx_sb = rpool.tile([128, NHEAD, S], BF16, tag="ctx")

                    # ------------ P1: q/k projections (feature-major out) -------
                    with tc.tile_pool(name="p1x", bufs=2) as xpool, \
                         tc.tile_pool(name="p1w", bufs=2) as wpool, \
                         tc.tile_pool(name="p1ps", bufs=8, space="PSUM") as pspool:
                        for (xin, win, bsb, dst) in (
                            (xq, wq, bq_sb, q_sb),
                            (xk, wk, bk_sb, k_sb),
                        ):
                            xall = xpool.tile([128, 8, 2, S], FP8, tag="x")
                            nc.sync.dma_start(
                                xall[:],
                                xin.rearrange(
                                    "(kt two p) n -> p kt two n", p=128, two=2
                                ),
                            )
                            for m in range(8):
                                wm = wpool.tile([128, 8, 2, 128], FP8, tag="w")
                                nc.sync.dma_start(
                                    wm[:],
                                    win[:, m * 128:(m + 1) * 128].rearrange(
                                        "(kt two p) m -> p kt two m", p=128, two=2
                                    ),
                                )
                                for n in range(4):
                                    ps = pspool.tile([128, 512], F32, tag="ps")
                                    for kt in range(8):
                                        nc.tensor.matmul(
                                            ps[:],
                                            wm[:, kt],
                                            xall[:, kt, :, n * 512:(n + 1) * 512],
                                            start=(kt == 0),
                                            stop=(kt == 7),
                                            perf_mode=DR,
                                        )
                                    nc.scalar.activation(
                                        dst[:, m, n * 512:(n + 1) * 512],
                                        ps[:],
                                        AF.Identity,
                                        bias=bsb[:, m:m + 1],
                                    )

                    # ------------ P2: v projection (natural [s, e_out]) ---------
                    if phases < 2:
                        return nc, 0
                    with tc.tile_pool(name="p2w", bufs=1) as wvpool, \
                         tc.tile_pool(name="p2x", bufs=2) as xvpool, \
                         tc.tile_pool(name="p2ps", bufs=8, space="PSUM") as pspool:
                        wvsb = wvpool.tile([128, 8, 2, EG], FP8, tag="wv")
                        nc.sync.dma_start(
                            wvsb[:],
                            wv.rearrange("(kt two p) m -> p kt two m", p=128, two=2),
                        )
                        for m in range(16):
                            xm = xvpool.tile([128, 8, 2, 128], FP8, tag="xv")
                            nc.sync.dma_start(
                                xm[:],
                                xv[:, m * 128:(m + 1) * 128].rearrange(
                                    "(kt two p) s -> p kt two s", p=128, two=2
                                ),
                            )
                            for n in range(2):
                                ps = pspool.tile([128, 512], F32, tag="psv")
                                for kt in range(8):
                                    nc.tensor.matmul(
                                        ps[:],
                                        xm[:, kt],
                                        wvsb[:, kt, :, n * 512:(n + 1) * 512],
                                        start=(kt == 0),
                                        stop=(kt == 7),
                                        perf_mode=DR,
                                    )
                                nc.vector.tensor_add(
                                    v_sb[:, m, n * 512:(n + 1) * 512],
                                    ps[:],
                                    bv_sb[:, n * 512:(n + 1) * 512],
                                )

                    # ------------ P3: attention, 1-deep software pipeline -------
                    if phases < 3:
                        return nc, 0
                    with tc.tile_pool(name="p3p", bufs=16) as ppool, \
                         tc.tile_pool(name="p3n", bufs=2) as npool, \
                         tc.tile_pool(name="p3s", bufs=4) as spool, \
                         tc.tile_pool(name="psS", bufs=2, space="PSUM") as psS, \
                         tc.tile_pool(name="psC", bufs=2, space="PSUM") as psC, \
                         tc.tile_pool(name="psZ", bufs=2, space="PSUM") as psZ:

                        def normalize(prev):
                            h, q0, _, ps_ctx, ps_z = prev
                            nsb = npool.tile([128, 512], BF16, tag="n")
                            nc.sync.dma_start(
                                nsb[:],
                                noi[h * 128:(h + 1) * 128, q0:q0 + 512],
                            )
                            rb_sb = spool.tile([128, 512], F32, tag="rb")
                            nc.vector.reciprocal(rb_sb[:], ps_z[:])
                            tmp = spool.tile([128, 512], BF16, tag="tmp")
                            nc.vector.tensor_mul(tmp[:], ps_ctx[:], rb_sb[:])
                            nc.vector.tensor_add(
                                ctx_sb[:, h, q0:q0 + 512], tmp[:], nsb[:]
                            )

                        prev = None
                        for h in range(NHEAD):
                            for qc in range(4):
                                q0 = qc * 512
                                cur_tiles = []
                                pc = pz = None
                                if prev is not None:
                                    # AV/Z chains of prev interleave with our
                                    # scores: one PSUM bank per chain.
                                    pc = psC.tile([128, 512], F32, tag="ctxps")
                                    pz = psZ.tile([128, 512], F32, tag="zps")
                                    prev = prev[:3] + (pc, pz)
                                for kt2 in range(8):
                                    ps2 = psS.tile([128, 2, 512], F32, tag="sps")
                                    psb = ppool.tile([128, 2, 512], FP8W, tag="p")
                                    for half in range(2):
                                        kc = kt2 * 2 + half
                                        nc.tensor.matmul(
                                            ps2[:, half],
                                            k_sb[:, h, kc * 128:(kc + 1) * 128],
                                            q_sb[:, h, q0:q0 + 512],
                                            start=True,
                                            stop=True,
                                        )
                                    if kt2 in ACT_KT2:
                                        nc.scalar.activation(
                                            psb[:], ps2[:], AF.Exp, scale=SCALE
                                        )
                                    else:
                                        nc.vector.tensor_scalar(
                                            psb[:].bitcast(INT8),
                                            ps2[:],
                                            EXP_A,
                                            EXP_B,
                                            ALU.mult,
                                            ALU.add,
                                        )
                                    cur_tiles.append(psb)
                                    if prev is not None:
                                        ph, pq0, ptiles, pc, pz = prev
                                        nc.tensor.matmul(
                                            pc[:],
                                            v_sb[:, 2 * kt2:2 * kt2 + 2,
                                                 ph * 128:(ph + 1) * 128],
                                            ptiles[kt2][:],
                                            start=(kt2 == 0),
                                            stop=(kt2 == 7),
                                            perf_mode=DR,
                                        )
                                        nc.tensor.matmul(
                                            pz[:],
                                            ones2[:],
                                            ptiles[kt2][:],
                                            start=(kt2 == 0),
                                            stop=(kt2 == 7),
                                            perf_mode=DR,
                                        )
                                if prev is not None:
                                    normalize(prev)
                                prev = (h, q0, cur_tiles, None, None)
                        # flush last iteration
                        ph, pq0, ptiles, _, _ = prev
                        pc = psC.tile([128, 512], F32, tag="ctxps")
                        pz = psZ.tile([128, 512], F32, tag="zps")
                        for kt2 in range(8):
                            nc.tensor.matmul(
                                pc[:],
                                v_sb[:, 2 * kt2:2 * kt2 + 2,
                                     ph * 128:(ph + 1) * 128],
                                ptiles[kt2][:],
                                start=(kt2 == 0),
                                stop=(kt2 == 7),
                                perf_mode=DR,
                            )
                            nc.tensor.matmul(
                                pz[:],
                                ones2[:],
                                ptiles[kt2][:],
                                start=(kt2 == 0),
                                stop=(kt2 == 7),
                                perf_mode=DR,
                            )
                        normalize((ph, pq0, ptiles, pc, pz))

                    # ------------ P4: out projection (bf16) ---------------------
                    if phases < 4:
                        return nc, 0
                    with tc.tile_pool(name="p4w", bufs=2) as wpool4, \
                         tc.tile_pool(name="p4o", bufs=4) as opool, \
                         tc.tile_pool(name="p4ps", bufs=8, space="PSUM") as pspool:
                        for m in range(16):
                            wosb = wpool4.tile([128, NHEAD, 128], BF16, tag="wo")
                            nc.sync.dma_start(
                                wosb[:],
                                wo[:, m * 128:(m + 1) * 128].rearrange(
                                    "(kt p) n -> p kt n", p=128
                                ),
                            )
                            for n in range(4):
                                ps = pspool.tile([128, 512], F32, tag="pso")
                                for kt in range(NHEAD):
                                    nc.tensor.matmul(
                                        ps[:],
                                        wosb[:, kt],
                                        ctx_sb[:, kt, n * 512:(n + 1) * 512],
                                        start=(kt == 0),
                                        stop=(kt == NHEAD - 1),
                                    )
                                osb = opool.tile([128, 512], BF16, tag="oo")
                                nc.scalar.copy(osb[:], ps[:])
                                nc.sync.dma_start(
                                    out[m * 128:(m + 1) * 128,
                                        n * 512:(n + 1) * 512],
                                    osb[:],
                                )
    n = split_excess_waits(nc)
    return nc, n


B = 4
NOISE_SCALE = 1.0 * math.sqrt(2.0 * math.log(1.25 / 1e-05)) / 1.0


def _make_in_maps(query, key_t, value, Wq, bq, Wk, bk, Wv, bv, Wo, bo, noise):
    import ml_dtypes

    E4 = ml_dtypes.float8_e4m3
    BF = ml_dtypes.bfloat16
    WqT = np.asarray(Wq, np.float32).T.astype(E4)
    WkT = np.asarray(Wk, np.float32).T.astype(E4)
    WvT = np.asarray(Wv, np.float32).T.astype(E4)
    WoT = np.asarray(Wo, np.float32).T.astype(BF)
    bq = np.asarray(bq, np.float32)
    bk = np.asarray(bk, np.float32)
    bv = np.asarray(bv, np.float32)
    xts = {}
    for b in range(B):
        xts[b] = (
            np.ascontiguousarray(np.asarray(query[b], np.float32).T).astype(E4),
            np.ascontiguousarray(np.asarray(key_t[b], np.float32).T).astype(E4),
            np.ascontiguousarray(np.asarray(value[b], np.float32).T).astype(E4),
        )
    in_maps = []
    for c in range(8):
        b, g = c // 2, c % 2
        cols = slice(g * EG, (g + 1) * EG)
        in_maps.append({
            "xq8": xts[b][0],
            "xk8": xts[b][1],
            "xv8": xts[b][2],
            "wq8": np.ascontiguousarray(WqT[:, cols]),
            "wk8": np.ascontiguousarray(WkT[:, cols]),
            "wv8": np.ascontiguousarray(WvT[:, cols]),
            "wob": np.ascontiguousarray(WoT[cols, :]),
            "bq2": np.ascontiguousarray(bq[cols].reshape(8, 128).T),
            "bk2": np.ascontiguousarray(bk[cols].reshape(8, 128).T),
            "bvb": np.ascontiguousarray(
                np.broadcast_to(bv[cols][None, :], (128, EG))
            ),
            "noiseT": np.ascontiguousarray(
                (np.asarray(noise[b], np.float32)[:, cols].T * NOISE_SCALE)
            ).astype(BF),
        })
    return in_maps


def kernel(**inputs) -> np.ndarray:
    from concourse.bass_utils import run_bass_kernel_spmd

    nc, _ = build_kernel_nc()
    in_maps = _make_in_maps(**inputs)
    res = run_bass_kernel_spmd(nc, in_maps, core_ids=list(range(8)))
    bo = np.asarray(inputs["bo"], np.float32)
    out = np.empty((B, S, E), np.float32)
    for b in range(B):
        p0 = res.results[2 * b]["outT"].astype(np.float32)
        p1 = res.results[2 * b + 1]["outT"].astype(np.float32)
        out[b] = (p0 + p1).T + bo[None, :]
    return out


# revision 13
# speedup vs baseline: 1.1073x; 1.0014x over previous
"""Trainium2 Bass kernel for nn_DPFlashAttention (B=4, S=2048, E=2048, H=16).

Sharding: 8 cores = 4 batches (data-parallel) x 2 head-groups (tensor-parallel
over heads). Core c handles batch c//2, heads (c%2)*8 .. (c%2)*8+8.

v3: N_out=512 everywhere + software-pipelined P3.
All DoubleRow matmuls emit a full PSUM bank (N_out=512, rhs free 1024) so the
256-col LDWEIGHTS (213ns @1.2GHz) hides under the ~241ns fill; at the old
N_out=256 the weight load dominated (213 vs 120ns).
P3 runs a 1-deep software pipeline: scores+exp of (h,qc) iter i interleave
with attn@V + Z(ones) matmuls of iter i-1 in PE program order, so the PE
never waits on exp. exp is split per kt2 pair across the Act engine (native
Exp -> e5m2) and DVE (bit-trick: e5m2 bits = trunc(z*4/ln2 + 60) as int8;
the uniform half-LSB truncation bias cancels in softmax normalization).
  P1  q/k projections fp8e4 DR, bias via Act Identity(bias) -> resident SBUF
  P2  v projection fp8e4 DR, bias via DVE tensor_add (per-column bias)
  P3  per (h,qc): 16 score MMs (fp8, K=128, N=512), 8 exp pairs (Act 5/DVE 3),
      8 AV MMs + 8 Z MMs (DR, N_out=512), normalize = DVE recip+mul,
      noise add -> resident bf16 ctx
  P4  out^T partial = Wo_shard @ (ctx + noise) in bf16, bf16 DMA out
Host: pre-transposes + pre-quantizes per-batch inputs (fp8e4) and weights,
pre-scales noise by the DP sigma (bf16), sums head-group partials,
transposes back, adds bo.
"""
import math
import sys

sys.path.insert(0, "/opt/trn_rl_repo")

import numpy as np

import concourse.bass as bass
import concourse.mybir as mybir
import concourse.tile as tile
from concourse.vector_clock import ScopedClock


class TileContextFixed(tile.TileContext):
    """This walrus build caps sync waits per instruction; split the closing
    drain's waits across single-wait NoOps (same engine => same semantics)."""

    def _drain_and_barrier(self, tick_clock, wait_clock):
        carrier = self.nc.sync.nop(nofuse=True, hint="drain_waits")
        wait_clock.add_sem_waits(
            carrier.ins, ScopedClock({None: tick_clock.global_clock})
        )
        si = carrier.ins.sync_info
        waits = list(si.on_wait) if si is not None else []
        if si is not None:
            si.on_wait[:] = waits[:1]
        for w in waits[1:]:
            n = self.nc.sync.nop(nofuse=True, hint="drain_waits")
            n.ins.sync_info = mybir.SyncInfo(on_wait=[w], on_update=[])
        self.nc.sync.drain()
        self.nc.all_engine_barrier()
        assert self.sems is not None
        popped = self.nc._tile_sem_poison_stack.pop()
        assert popped is self._sem_poison
        self.nc.clear_and_free_semaphores(list(self.sems.allocated().values()))
        self.nc.all_engine_barrier()


def split_excess_waits(nc, opcodes=None, cap=1):
    """Hoist waits beyond `cap` onto same-engine NoOps placed just before the
    instruction; engine queues execute in order so blocking is preserved."""
    n_split = 0
    for fn in nc.m.functions:
        for blk in fn.blocks:
            new = []
            for inst in blk.instructions:
                si = inst.sync_info
                if (
                    (opcodes is None or inst.opcode in opcodes)
                    and si is not None
                    and len(si.on_wait) > cap
                ):
                    waits = list(si.on_wait)
                    for j, w in enumerate(waits[cap:]):
                        nop = mybir.InstNoOp(
                            name=f"{inst.name}-w{j}", engine=inst.engine
                        )
                        nop.sync_info = mybir.SyncInfo(on_wait=[w], on_update=[])
                        new.append(nop)
                        n_split += 1
                    si.on_wait[:] = waits[:cap]
                new.append(inst)
            blk.instructions[:] = new
    return n_split

def _lw_sig(inst):
    ap = inst.ins[0]
    return (
        ap.memref,
        ap.offset,
        tuple(tuple(p) for p in ap.ap),
        str(ap.dtype),
        str(inst.perf_mode),
        bool(getattr(inst, "is_transpose", False)),
        tuple(inst.tile_position or ()),
        tuple(inst.tile_size or ()),
    )


def dedup_ldweights(nc):
    """Drop an InstLdweights identical to the previous one on the PE queue
    when only plain Matmults ran in between (the PE weight buffer still
    holds those weights). Sync waits/updates of dropped LWs merge into the
    next instruction (later = conservative)."""
    n_drop = 0
    for fn in nc.m.functions:
        for blk in fn.blocks:
            new = []
            last_sig = None
            pending_sync = []
            for inst in blk.instructions:
                if inst.engine != mybir.EngineType.PE:
                    new.append(inst)
                    continue
                if inst.opcode == "Ldweights":
                    sig = _lw_sig(inst)
                    if sig == last_sig:
                        if inst.sync_info is not None:
                            pending_sync.append(inst.sync_info)
                        n_drop += 1
                        continue
                    last_sig = sig
                elif inst.opcode == "Matmult":
                    if getattr(inst, "is_transpose", False):
                        last_sig = None
                elif inst.opcode != "NoOp":
                    last_sig = None
                if pending_sync:
                    si = inst.sync_info
                    if si is None:
                        si = mybir.SyncInfo(on_wait=[], on_update=[])
                        inst.sync_info = si
                    for ps in pending_sync:
                        si.on_wait.extend(ps.on_wait)
                        si.on_update.extend(ps.on_update)
                    pending_sync = []
                new.append(inst)
            assert not pending_sync
            blk.instructions[:] = new
    return n_drop


F32 = mybir.dt.float32
F32R = mybir.dt.float32r
BF16 = mybir.dt.bfloat16
FP8 = mybir.dt.float8e4
FP8W = mybir.dt.float8e5   # attn weights: e5m2 spans exp(+-9) w/o subnormals
INT8 = mybir.dt.int8
AF = mybir.ActivationFunctionType
ALU = mybir.AluOpType
DR = mybir.MatmulPerfMode.DoubleRow

S = 2048
E = 2048
EG = 1024          # per-core e_out shard (8 heads x 128)
D = 128
NHEAD = 8          # heads per core
SCALE = 1.0 / math.sqrt(128.0)
# DVE bit-trick exp -> e5m2 bits: trunc(z*(4/ln2)*SCALE + 60) = exp(z*SCALE)
# in e5m2 (x 2^-1/8 uniform bias that cancels in softmax normalization).
EXP_A = SCALE * 4.0 / math.log(2.0)
EXP_B = 60.0
ACT_KT2 = (0, 2, 4, 6, 7)  # exp pairs on Act engine; rest on DVE


def build_kernel_nc(phases=4, repeat=1):
    nc = bass.Bass()

    # All inputs are pre-arranged on the host into the exact on-chip layout
    # so every DMA is a contiguous per-partition copy (~128 descriptors);
    # rearranged DRAM APs cost ~2us of SP-sequencer descriptor generation
    # per load and serialize the whole DMA queue.
    xq = nc.dram_tensor("xq8", [128, 8, 2, S], FP8, kind="ExternalInput")
    xk = nc.dram_tensor("xk8", [128, 8, 2, S], FP8, kind="ExternalInput")
    xv = nc.dram_tensor("xv8", [16, 128, 8, 2, 128], FP8, kind="ExternalInput")
    wq = nc.dram_tensor("wq8", [8, 128, 8, 2, 128], FP8, kind="ExternalInput")
    wk = nc.dram_tensor("wk8", [8, 128, 8, 2, 128], FP8, kind="ExternalInput")
    wv = nc.dram_tensor("wv8", [128, 8, 2, EG], FP8, kind="ExternalInput")
    wo = nc.dram_tensor("wob", [16, 128, NHEAD, 128], BF16, kind="ExternalInput")
    bq = nc.dram_tensor("bq2", [128, 8], F32, kind="ExternalInput")
    bk = nc.dram_tensor("bk2", [128, 8], F32, kind="ExternalInput")
    bv = nc.dram_tensor("bvb", [128, EG], F32, kind="ExternalInput")
    noi = nc.dram_tensor("noiseT", [EG, S], BF16, kind="ExternalInput")
    out = nc.dram_tensor("outT", [E, S], BF16, kind="ExternalOutput")

    for _rep in range(repeat):
        with TileContextFixed(nc) as tc, \
             nc.allow_low_precision(reason="fp8 attention path is intended"):
            with tc.tile_pool(name="const", bufs=1) as cpool:
                bq_sb = cpool.tile([128, 8], F32, tag="bq")
                nc.sync.dma_start(bq_sb[:], bq[:])
                bk_sb = cpool.tile([128, 8], F32, tag="bk")
                nc.sync.dma_start(bk_sb[:], bk[:])
                bv_sb = cpool.tile([128, EG], F32, tag="bv")
                nc.sync.dma_start(bv_sb[:], bv[:])
                ones2 = cpool.tile([128, 2, 128], FP8W, tag="ones2")
                nc.vector.memset(ones2[:], 1.0)

                with tc.tile_pool(name="res", bufs=1) as rpool:
                    q_sb = rpool.tile([128, NHEAD, S], FP8, tag="q")
                    k_sb = rpool.tile([128, NHEAD, S], FP8, tag="k")
                    v_sb = rpool.tile([128, 16, EG], FP8, tag="v")
                    ctx_sb = rpool.tile([128, NHEAD, S], BF16, tag="ctx")

                    # ------------ P1: q/k projections (feature-major out) -------
                    # q loads issue on the SP queue, k loads on the Act queue:
                    # both transfer concurrently, and k is resident before the
                    # PE finishes the q projection. x streams in 8 kt-chunks so
                    # the first matmul starts ~1.6us in instead of ~13us.
                    with tc.tile_pool(name="p1x", bufs=16) as xpool, \
                         tc.tile_pool(name="p1wq", bufs=2) as wpoolq, \
                         tc.tile_pool(name="p1wk", bufs=2) as wpoolk, \
                         tc.tile_pool(name="p1ps", bufs=8, space="PSUM") as pspool:
                        jobs = []
                        for (xin, win, bsb, dst, eng, wpool) in (
                            (xq, wq, bq_sb, q_sb, nc.sync, wpoolq),
                            (xk, wk, bk_sb, k_sb, nc.scalar, wpoolk),
                        ):
                            wms = []
                            for m in range(2):
                                wm = wpool.tile([128, 8, 2, 128], FP8, tag="w")
                                eng.dma_start(wm[:], win[m])
                                wms.append(wm)
                            xcs = []
                            for kt in range(8):
                                xc = xpool.tile([128, 2, S], FP8, tag="x")
                                eng.dma_start(xc[:], xin[:, kt])
                                xcs.append(xc)
                            jobs.append((win, bsb, dst, eng, wpool, wms, xcs))
                        for (win, bsb, dst, eng, wpool, wms, xcs) in jobs:
                            for m in range(8):
                                if m < 2:
                                    wm = wms[m]
                                else:
                                    wm = wpool.tile([128, 8, 2, 128], FP8, tag="w")
                                    eng.dma_start(wm[:], win[m])
                                for n in range(4):
                                    ps = pspool.tile([128, 512], F32, tag="ps")
                                    for kt in range(8):
                                        nc.tensor.matmul(
                                            ps[:],
                                            wm[:, kt],
                                            xcs[kt][:, :, n * 512:(n + 1) * 512],
                                            start=(kt == 0),
                                            stop=(kt == 7),
                                            perf_mode=DR,
                                        )
                                    nc.scalar.activation(
                                        dst[:, m, n * 512:(n + 1) * 512],
                                        ps[:],
                                        AF.Identity,
                                        bias=bsb[:, m:m + 1],
                                    )

                    # ------------ P2: v projection (natural [s, e_out]) ---------
                    if phases < 2:
                        continue
                    with tc.tile_pool(name="p2w", bufs=1) as wvpool, \
                         tc.tile_pool(name="p2x", bufs=2) as xvpool, \
                         tc.tile_pool(name="p2ps", bufs=8, space="PSUM") as pspool:
                        wvsb = wvpool.tile([128, 8, 2, EG], FP8, tag="wv")
                        nc.scalar.dma_start(wvsb[:], wv[:])
                        for m in range(16):
                            xm = xvpool.tile([128, 8, 2, 128], FP8, tag="xv")
                            nc.sync.dma_start(xm[:], xv[m])
                            for n in range(2):
                                ps = pspool.tile([128, 512], F32, tag="psv")
                                for kt in range(8):
                                    nc.tensor.matmul(
                                        ps[:],
                                        xm[:, kt],
                                        wvsb[:, kt, :, n * 512:(n + 1) * 512],
                                        start=(kt == 0),
                                        stop=(kt == 7),
                                        perf_mode=DR,
                                    )
                                nc.vector.tensor_add(
                                    v_sb[:, m, n * 512:(n + 1) * 512],
                                    ps[:],
                                    bv_sb[:, n * 512:(n + 1) * 512],
                                )

                    # ------------ P3: attention, 1-deep software pipeline -------
                    if phases < 3:
                        continue
                    wpool4 = tc.alloc_tile_pool(name="p4w", bufs=2)
                    wosb_pre = []
                    for m in range(2):
                        w4 = wpool4.tile([128, NHEAD, 128], BF16, tag="wo")
                        nc.scalar.dma_start(w4[:], wo[m])
                        wosb_pre.append(w4)
                    with tc.tile_pool(name="p3p", bufs=16) as ppool, \
                         tc.tile_pool(name="p3n", bufs=2) as npool, \
                         tc.tile_pool(name="p3s", bufs=4) as spool, \
                         tc.tile_pool(name="psS", bufs=2, space="PSUM") as psS, \
                         tc.tile_pool(name="psC", bufs=2, space="PSUM") as psC, \
                         tc.tile_pool(name="psZ", bufs=2, space="PSUM") as psZ:

                        def normalize(prev):
                            h, q0, _, ps_ctx, ps_z = prev
                            nsb = npool.tile([128, 512], BF16, tag="n")
                            nc.sync.dma_start(
                                nsb[:],
                                noi[h * 128:(h + 1) * 128, q0:q0 + 512],
                            )
                            rb_sb = spool.tile([128, 512], F32, tag="rb")
                            nc.vector.reciprocal(rb_sb[:], ps_z[:])
                            tmp = spool.tile([128, 512], BF16, tag="tmp")
                            nc.vector.tensor_mul(tmp[:], ps_ctx[:], rb_sb[:])
                            nc.vector.tensor_add(
                                ctx_sb[:, h, q0:q0 + 512], tmp[:], nsb[:]
                            )

                        prev = None
                        for h in range(NHEAD):
                            for qc in range(4):
                                q0 = qc * 512
                                cur_tiles = []
                                pc = pz = None
                                if prev is not None:
                                    # AV/Z chains of prev interleave with our
                                    # scores: one PSUM bank per chain.
                                    pc = psC.tile([128, 512], F32, tag="ctxps")
                                    pz = psZ.tile([128, 512], F32, tag="zps")
                                    prev = prev[:3] + (pc, pz)
                                for kt2 in range(8):
                                    ps2 = psS.tile([128, 2, 512], F32, tag="sps")
                                    psb = ppool.tile([128, 2, 512], FP8W, tag="p")
                                    for half in range(2):
                                        kc = kt2 * 2 + half
                                        nc.tensor.matmul(
                                            ps2[:, half],
                                            k_sb[:, h, kc * 128:(kc + 1) * 128],
                                            q_sb[:, h, q0:q0 + 512],
                                            start=True,
                                            stop=True,
                                        )
                                    if kt2 in ACT_KT2:
                                        nc.scalar.activation(
                                            psb[:], ps2[:], AF.Exp, scale=SCALE
                                        )
                                    else:
                                        nc.vector.tensor_scalar(
                                            psb[:].bitcast(INT8),
                                            ps2[:],
                                            EXP_A,
                                            EXP_B,
                                            ALU.mult,
                                            ALU.add,
                                        )
                                    cur_tiles.append(psb)
                                    if prev is not None:
                                        ph, pq0, ptiles, pc, pz = prev
                                        nc.tensor.matmul(
                                            pc[:],
                                            v_sb[:, 2 * kt2:2 * kt2 + 2,
                                                 ph * 128:(ph + 1) * 128],
                                            ptiles[kt2][:],
                                            start=(kt2 == 0),
                                            stop=(kt2 == 7),
                                            perf_mode=DR,
                                        )
                                        nc.tensor.matmul(
                                            pz[:],
                                            ones2[:],
                                            ptiles[kt2][:],
                                            start=(kt2 == 0),
                                            stop=(kt2 == 7),
                                            perf_mode=DR,
                                        )
                                if prev is not None:
                                    normalize(prev)
                                prev = (h, q0, cur_tiles, None, None)
                        # flush last iteration
                        ph, pq0, ptiles, _, _ = prev
                        pc = psC.tile([128, 512], F32, tag="ctxps")
                        pz = psZ.tile([128, 512], F32, tag="zps")
                        for kt2 in range(8):
                            nc.tensor.matmul(
                                pc[:],
                                v_sb[:, 2 * kt2:2 * kt2 + 2,
                                     ph * 128:(ph + 1) * 128],
                                ptiles[kt2][:],
                                start=(kt2 == 0),
                                stop=(kt2 == 7),
                                perf_mode=DR,
                            )
                            nc.tensor.matmul(
                                pz[:],
                                ones2[:],
                                ptiles[kt2][:],
                                start=(kt2 == 0),
                                stop=(kt2 == 7),
                                perf_mode=DR,
                            )
                        normalize((ph, pq0, ptiles, pc, pz))

                    # ------------ P4: out projection (bf16) ---------------------
                    if phases < 4:
                        wpool4.release()
                        continue
                    with tc.tile_pool(name="p4o", bufs=4) as opool, \
                         tc.tile_pool(name="p4ps", bufs=8, space="PSUM") as pspool:
                        for m in range(16):
                            if m < 2:
                                wosb = wosb_pre[m]
                            else:
                                wosb = wpool4.tile(
                                    [128, NHEAD, 128], BF16, tag="wo"
                                )
                                nc.scalar.dma_start(wosb[:], wo[m])
                            for n in range(4):
                                ps = pspool.tile([128, 512], F32, tag="pso")
                                for kt in range(NHEAD):
                                    nc.tensor.matmul(
                                        ps[:],
                                        wosb[:, kt],
                                        ctx_sb[:, kt, n * 512:(n + 1) * 512],
                                        start=(kt == 0),
                                        stop=(kt == NHEAD - 1),
                                    )
                                osb = opool.tile([128, 512], BF16, tag="oo")
                                nc.scalar.copy(osb[:], ps[:])
                                nc.sync.dma_start(
                                    out[m * 128:(m + 1) * 128,
                                        n * 512:(n + 1) * 512],
                                    osb[:],
                                )
                    wpool4.release()
    dedup_ldweights(nc)
    n = split_excess_waits(nc)
    return nc, n


B = 4
NOISE_SCALE = 1.0 * math.sqrt(2.0 * math.log(1.25 / 1e-05)) / 1.0


def _x_layout(xT):
    """[E, S] feature-major -> [128, 8, 2, S] DoubleRow stream layout
    (feature e = kt*256 + two*128 + p)."""
    return np.ascontiguousarray(
        xT.reshape(8, 2, 128, S).transpose(2, 0, 1, 3)
    )


def _xv_layout(xT):
    """[E, S] -> [16, 128, 8, 2, 128]: per seq-block m the stationary
    tile [p, kt, two, s]."""
    return np.ascontiguousarray(
        xT.reshape(8, 2, 128, 16, 128).transpose(3, 2, 0, 1, 4)
    )


def _w_layout(wT):
    """[E, EG_cols] -> [n_m, 128, 8, 2, 128] per-m weight tiles."""
    n_m = wT.shape[1] // 128
    return np.ascontiguousarray(
        wT.reshape(8, 2, 128, n_m, 128).transpose(3, 2, 0, 1, 4)
    )


def _wv_layout(wT):
    """[E, EG] -> [128, 8, 2, EG] streaming weight layout."""
    return np.ascontiguousarray(wT.reshape(8, 2, 128, EG).transpose(2, 0, 1, 3))


def _wo_layout(woT):
    """[EG, E] -> [16, 128, 8, 128] per-m stationary tiles [p, kt, n]."""
    return np.ascontiguousarray(
        woT.reshape(8, 128, 16, 128).transpose(2, 1, 0, 3)
    )


def _make_in_maps(query, key_t, value, Wq, bq, Wk, bk, Wv, bv, Wo, bo, noise):
    import ml_dtypes

    E4 = ml_dtypes.float8_e4m3
    BF = ml_dtypes.bfloat16
    WqT = np.asarray(Wq, np.float32).T.astype(E4)
    WkT = np.asarray(Wk, np.float32).T.astype(E4)
    WvT = np.asarray(Wv, np.float32).T.astype(E4)
    WoT = np.asarray(Wo, np.float32).T.astype(BF)
    bq = np.asarray(bq, np.float32)
    bk = np.asarray(bk, np.float32)
    bv = np.asarray(bv, np.float32)
    xts = {}
    for b in range(B):
        xts[b] = (
            _x_layout(np.asarray(query[b], np.float32).T.astype(E4)),
            _x_layout(np.asarray(key_t[b], np.float32).T.astype(E4)),
            _xv_layout(np.asarray(value[b], np.float32).T.astype(E4)),
        )
    in_maps = []
    for c in range(8):
        b, g = c // 2, c % 2
        cols = slice(g * EG, (g + 1) * EG)
        in_maps.append({
            "xq8": xts[b][0],
            "xk8": xts[b][1],
            "xv8": xts[b][2],
            "wq8": _w_layout(WqT[:, cols]),
            "wk8": _w_layout(WkT[:, cols]),
            "wv8": _wv_layout(WvT[:, cols]),
            "wob": _wo_layout(WoT[cols, :]),
            "bq2": np.ascontiguousarray(bq[cols].reshape(8, 128).T),
            "bk2": np.ascontiguousarray(bk[cols].reshape(8, 128).T),
            "bvb": np.ascontiguousarray(
                np.broadcast_to(bv[cols][None, :], (128, EG))
            ),
            "noiseT": np.ascontiguousarray(
                (np.asarray(noise[b], np.float32)[:, cols].T * NOISE_SCALE)
            ).astype(BF),
        })
    return in_maps


def kernel(**inputs) -> np.ndarray:
    from concourse.bass_utils import run_bass_kernel_spmd

    nc, _ = build_kernel_nc()
    in_maps = _make_in_maps(**inputs)
    res = run_bass_kernel_spmd(nc, in_maps, core_ids=list(range(8)))
    bo = np.asarray(inputs["bo"], np.float32)
    out = np.empty((B, S, E), np.float32)
    for b in range(B):
        p0 = res.results[2 * b]["outT"].astype(np.float32)
        p1 = res.results[2 * b + 1]["outT"].astype(np.float32)
        out[b] = (p0 + p1).T + bo[None, :]
    return out
